# revision 7
# baseline (speedup 1.0000x reference)
"""Trainium2 Bass kernel for nn_BatchTrainableButterfly.

The reference applies, per mesh-batch b, a trainable butterfly network
(10 levels of phase shifters + 2x2 directional couplers with butterfly
permutations, plus a final phase layer and bit-reversals) to every token
row x[n, :].  For fixed phases the whole network is a linear map on
C^1024, so out[b] = x @ W_b with W_b = network_b(I_1024) — a 1024x1024
complex64 matrix that is cheap to build on host (O(L^2 log L) total).

Device work per core (8 cores = 4 mesh-batches x 2 token halves):
  out_half[b] = x_half @ W_b as real fp32r matmuls on TensorE:
    re = xr@Wr + xi@(-Wi),  im = xr@Wi + xi@Wr
x arrives token-major, so each 128-token tile is transposed on the PE
(L on partitions) to serve as the matmul stationary operand; results
accumulate in PSUM, are interleaved re/im into SBUF and DMA'd out as
complex64-compatible rows.
"""

import math

import numpy as np

import concourse.tile as tile
from concourse import bacc, bass, mybir
from concourse.bass_utils import run_bass_kernel_spmd
from concourse.masks import make_identity

P = 128          # partitions
L = 1024         # butterfly length
N_TOKENS = 4096
MESH_BATCH = 4
N_CORES = 8
T = (N_TOKENS * MESH_BATCH) // N_CORES  # 2048 token-rows per core
NT = T // P      # 16 token tiles per core
KC = L // P      # 8 contraction chunks
NLEV = int(math.log2(L))  # 10

F32 = mybir.dt.float32
F32R = mybir.dt.float32r

TRACE = False
LAST_RESULTS = None

# ----------------------------------------------------------------------
# Host side: build the per-batch transfer matrices from the phases.
# ----------------------------------------------------------------------


def _bitrev(n):
    m = int(math.log2(n))
    perm = np.arange(n).reshape(n, 1)
    for _ in range(m):
        n1 = perm.shape[0] // 2
        perm = np.hstack((perm[:n1], perm[n1:]))
    return perm.squeeze(0)


def _forward_indices(length):
    idx = []
    ar = np.arange(length)
    for level in range(int(math.log2(length)) - 1):
        bs = 2 ** (level + 2)
        ind = ar.reshape(-1, length // bs, 2, bs // 2).transpose(0, 1, 3, 2)
        idx.append(ind.reshape(-1))
    return idx


def _build_W(phases):
    """phases (B, NLEV+1, L//2, 2) -> W (B, L, L) complex64 with out = x @ W."""
    B = phases.shape[0]
    br = _bitrev(L)
    fidx = _forward_indices(L)
    dc = np.array([[1.0, 1.0j], [1.0j, 1.0]], dtype=np.complex64)

    x = np.broadcast_to(np.eye(L, dtype=np.complex64), (B, L, L)).copy()
    x = x[..., br]
    for level in range(NLEV):
        x = x.reshape(B, L, L // 2, 2)
        ph = phases[:, level : level + 1, :, :]            # (B, 1, L//2, 2)
        x = x * np.exp(1j * ph.astype(np.complex64))
        x = x @ dc
        x = x.reshape(B, L, L)
        if level < NLEV - 1:
            x = x[..., fidx[level]]
    ph = phases[:, NLEV - 1 : NLEV, :, :].reshape(B, 1, L)
    x = x * np.exp(1j * ph.astype(np.complex64))
    x = x[..., br]
    return (x / np.float32(np.sqrt(L))).astype(np.complex64)


# ----------------------------------------------------------------------
# Device side: complex matmul kernel (SPMD, one (batch, half) per core).
# ----------------------------------------------------------------------

_CACHED_NC = None


def _build_program():
    nc = bacc.Bacc(
        "TRN2", target_bir_lowering=False, debug=False, num_devices=N_CORES
    )

    xr_d = nc.declare_dram_parameter("xr", [T, L], F32, isOutput=False)
    xi_d = nc.declare_dram_parameter("xi", [T, L], F32, isOutput=False)
    wr_d = nc.declare_dram_parameter("wr", [L, L], F32R, isOutput=False)
    wi_d = nc.declare_dram_parameter("wi", [L, L], F32R, isOutput=False)
    nwi_d = nc.declare_dram_parameter("nwi", [L, L], F32R, isOutput=False)
    out_d = nc.declare_dram_parameter("out", [T, 2 * L], F32, isOutput=True)

    with tile.TileContext(nc) as tc:
        with (
            tc.tile_pool(name="const", bufs=1) as const_pool,
            tc.tile_pool(name="w", bufs=1) as w_pool,
            tc.tile_pool(name="x", bufs=3) as x_pool,
            tc.tile_pool(name="xt", bufs=2) as xt_pool,
            tc.tile_pool(name="osb", bufs=3) as o_pool,
            tc.tile_pool(name="tp", bufs=2, space=bass.MemorySpace.PSUM) as tp_pool,
            tc.tile_pool(name="acc", bufs=6, space=bass.MemorySpace.PSUM) as acc_pool,
        ):
            ident = const_pool.tile([P, P], F32)
            make_identity(nc, ident[:])

            # Stream W into SBUF once: per k-chunk tiles (P x L), natural layout
            # (partition = contraction row within chunk, free = output column).
            w_sb = {}
            for nm, dram in (("wr", wr_d), ("wi", wi_d), ("nwi", nwi_d)):
                for k in range(KC):
                    t_ = w_pool.tile([P, L], F32R, tag=f"{nm}{k}")
                    nc.sync.dma_start(out=t_[:], in_=dram[k * P : (k + 1) * P, :])
                    w_sb[nm, k] = t_

            for t in range(NT):
                rows = slice(t * P, (t + 1) * P)
                xr_rows = x_pool.tile([P, L], F32, tag="xr_rows")
                xi_rows = x_pool.tile([P, L], F32, tag="xi_rows")
                nc.sync.dma_start(out=xr_rows[:], in_=xr_d[rows, :])
                nc.sync.dma_start(out=xi_rows[:], in_=xi_d[rows, :])

                # Transpose the token tile: xT chunks live at
                # xT[:, k*P:(k+1)*P] = x_rows[:, k*P:(k+1)*P].T
                xrT = xt_pool.tile([P, L], F32R, tag="xrT")
                xiT = xt_pool.tile([P, L], F32R, tag="xiT")
                for src, dst in ((xr_rows, xrT), (xi_rows, xiT)):
                    for g in range(2):
                        tp = tp_pool.tile([P, 4 * P], F32, tag="tp")
                        for j in range(4):
                            k = g * 4 + j
                            nc.tensor.transpose(
                                tp[:, j * P : (j + 1) * P],
                                src[:, k * P : (k + 1) * P],
                                ident[:],
                            )
                        nc.scalar.copy(dst[:, g * 4 * P : (g + 1) * 4 * P], tp[:])

                # Accumulate the four real matmul outputs.
                #   re_n = sum_k xrT_k @ wr_k[n] + xiT_k @ nwi_k[n]
                #   im_n = sum_k xrT_k @ wi_k[n] + xiT_k @ wr_k[n]
                out_sb = o_pool.tile([P, L, 2], F32, tag="out_sb")
                for n in range(2):
                    ncol = slice(n * 512, (n + 1) * 512)
                    acc_re = acc_pool.tile([P, 512], F32, tag="acc")
                    acc_im = acc_pool.tile([P, 512], F32, tag="acc")
                    for k in range(KC):
                        xrT_k = xrT[:, k * P : (k + 1) * P]
                        xiT_k = xiT[:, k * P : (k + 1) * P]
                        first = k == 0
                        last = k == KC - 1
                        nc.tensor.matmul(
                            acc_re[:], xrT_k, w_sb["wr", k][:, ncol],
                            start=first, stop=False,
                        )
                        nc.tensor.matmul(
                            acc_re[:], xiT_k, w_sb["nwi", k][:, ncol],
                            start=False, stop=last,
                        )
                        nc.tensor.matmul(
                            acc_im[:], xrT_k, w_sb["wi", k][:, ncol],
                            start=first, stop=False,
                        )
                        nc.tensor.matmul(
                            acc_im[:], xiT_k, w_sb["wr", k][:, ncol],
                            start=False, stop=last,
                        )
                    # Interleave re/im into complex64 memory order.
                    nc.vector.tensor_copy(out_sb[:, n * 512 : (n + 1) * 512, 0], acc_re[:])
                    nc.vector.tensor_copy(out_sb[:, n * 512 : (n + 1) * 512, 1], acc_im[:])

                nc.sync.dma_start(out=out_d[rows, :], in_=out_sb[:])

    nc.compile()
    return nc


def kernel(x_re: np.ndarray, x_im: np.ndarray, phases: np.ndarray) -> np.ndarray:
    global _CACHED_NC, LAST_RESULTS

    x_re = np.ascontiguousarray(x_re, dtype=np.float32)
    x_im = np.ascontiguousarray(x_im, dtype=np.float32)
    phases = np.ascontiguousarray(phases, dtype=np.float32)

    W = _build_W(phases)                      # (B, L, L) complex64
    Wr = np.ascontiguousarray(W.real, dtype=np.float32)
    Wi = np.ascontiguousarray(W.imag, dtype=np.float32)
    nWi = np.ascontiguousarray(-Wi)

    if _CACHED_NC is None:
        _CACHED_NC = _build_program()
    nc = _CACHED_NC

    half = N_TOKENS // 2
    in_maps = []
    for c in range(N_CORES):
        b, h = c // 2, c % 2
        in_maps.append(
            {
                "xr": x_re[h * half : (h + 1) * half],
                "xi": x_im[h * half : (h + 1) * half],
                "wr": Wr[b],
                "wi": Wi[b],
                "nwi": nWi[b],
            }
        )

    res = run_bass_kernel_spmd(nc, in_maps, list(range(N_CORES)), trace=TRACE)
    LAST_RESULTS = res

    out = np.empty((MESH_BATCH, N_TOKENS, L), dtype=np.complex64)
    for c in range(N_CORES):
        b, h = c // 2, c % 2
        out[b, h * half : (h + 1) * half] = (
            res.results[c]["out"].view(np.complex64).reshape(half, L)
        )
    return out


# revision 14
# speedup vs baseline: 1.0513x; 1.0513x over previous
"""Trainium2 Bass kernel for nn_BatchTrainableButterfly.

The reference applies, per mesh-batch b, a trainable butterfly network
(10 levels of phase shifters + 2x2 directional couplers with butterfly
permutations, plus a final phase layer and bit-reversals) to every token
row x[n, :].  For fixed phases the whole network is a linear map on
C^1024, so out[b] = x @ W_b with W_b = network_b(I_1024) — a 1024x1024
complex64 matrix that is cheap to build on host (O(L^2 log L) total).

Device work per core (8 cores = 4 mesh-batches x 2 token halves):
  out_half[b] = x_half @ W_b as real fp32r matmuls on TensorE:
    re = xr@Wr + xi@(-Wi),  im = xr@Wi + xi@Wr
x arrives token-major, so each 128-token tile is transposed on the PE
(L on partitions) to serve as the matmul stationary operand; results
accumulate in PSUM, are interleaved re/im into SBUF and DMA'd out as
complex64-compatible rows.
"""

import math

import numpy as np

import concourse.tile as tile
from concourse import bacc, bass, mybir
from concourse.bass_utils import run_bass_kernel_spmd
from concourse.masks import make_identity

P = 128          # partitions
L = 1024         # butterfly length
N_TOKENS = 4096
MESH_BATCH = 4
N_CORES = 8
T = (N_TOKENS * MESH_BATCH) // N_CORES  # 2048 token-rows per core
NT = T // P      # 16 token tiles per core
KC = L // P      # 8 contraction chunks
NLEV = int(math.log2(L))  # 10

F32 = mybir.dt.float32
F32R = mybir.dt.float32r

TRACE = False
LAST_RESULTS = None

# ----------------------------------------------------------------------
# Host side: build the per-batch transfer matrices from the phases.
# ----------------------------------------------------------------------


def _bitrev(n):
    m = int(math.log2(n))
    perm = np.arange(n).reshape(n, 1)
    for _ in range(m):
        n1 = perm.shape[0] // 2
        perm = np.hstack((perm[:n1], perm[n1:]))
    return perm.squeeze(0)


def _forward_indices(length):
    idx = []
    ar = np.arange(length)
    for level in range(int(math.log2(length)) - 1):
        bs = 2 ** (level + 2)
        ind = ar.reshape(-1, length // bs, 2, bs // 2).transpose(0, 1, 3, 2)
        idx.append(ind.reshape(-1))
    return idx


def _build_W(phases):
    """phases (B, NLEV+1, L//2, 2) -> W (B, L, L) complex64 with out = x @ W."""
    B = phases.shape[0]
    br = _bitrev(L)
    fidx = _forward_indices(L)
    dc = np.array([[1.0, 1.0j], [1.0j, 1.0]], dtype=np.complex64)

    x = np.broadcast_to(np.eye(L, dtype=np.complex64), (B, L, L)).copy()
    x = x[..., br]
    for level in range(NLEV):
        x = x.reshape(B, L, L // 2, 2)
        ph = phases[:, level : level + 1, :, :]            # (B, 1, L//2, 2)
        x = x * np.exp(1j * ph.astype(np.complex64))
        x = x @ dc
        x = x.reshape(B, L, L)
        if level < NLEV - 1:
            x = x[..., fidx[level]]
    ph = phases[:, NLEV - 1 : NLEV, :, :].reshape(B, 1, L)
    x = x * np.exp(1j * ph.astype(np.complex64))
    x = x[..., br]
    return (x / np.float32(np.sqrt(L))).astype(np.complex64)


# ----------------------------------------------------------------------
# Device side: complex matmul kernel (SPMD, one (batch, half) per core).
# ----------------------------------------------------------------------

_CACHED_NC = None


def _build_program():
    nc = bacc.Bacc(
        "TRN2", target_bir_lowering=False, debug=False, num_devices=N_CORES
    )

    xr_d = nc.declare_dram_parameter("xr", [T, L], F32, isOutput=False)
    xi_d = nc.declare_dram_parameter("xi", [T, L], F32, isOutput=False)
    wr_d = nc.declare_dram_parameter("wr", [L, L], F32R, isOutput=False)
    wi_d = nc.declare_dram_parameter("wi", [L, L], F32R, isOutput=False)
    nwi_d = nc.declare_dram_parameter("nwi", [L, L], F32R, isOutput=False)
    out_d = nc.declare_dram_parameter("out", [T, 2 * L], F32, isOutput=True)

    with tile.TileContext(nc) as tc:
        with (
            tc.tile_pool(name="const", bufs=1) as const_pool,
            tc.tile_pool(name="w", bufs=1) as w_pool,
            tc.tile_pool(name="x", bufs=3) as x_pool,
            tc.tile_pool(name="xt", bufs=2) as xt_pool,
            tc.tile_pool(name="osb", bufs=3) as o_pool,
            tc.tile_pool(name="ps", bufs=8, space=bass.MemorySpace.PSUM) as ps_pool,
        ):
            ident = const_pool.tile([P, P], F32)
            make_identity(nc, ident[:])

            # Warm the PE HAM while W streams in: dummy transposes keep the
            # tensor engine busy >3.4us so it reaches full clock before the
            # real matmuls start.
            warm = ps_pool.tile([P, 4 * P], F32, tag="ps")
            for _ in range(12):
                for j in range(4):
                    nc.tensor.transpose(
                        warm[:, j * P : (j + 1) * P], ident[:], ident[:]
                    )

            # Stream W into SBUF once: per k-chunk tiles (P x L), natural layout
            # (partition = contraction row within chunk, free = output column).
            # k-major order so the first token tile's accumulation can start
            # after only a few chunks have landed.
            w_sb = {}
            for k in range(KC):
                for nm, dram in (("wr", wr_d), ("wi", wi_d), ("nwi", nwi_d)):
                    t_ = w_pool.tile([P, L], F32R, tag=f"{nm}{k}")
                    nc.sync.dma_start(out=t_[:], in_=dram[k * P : (k + 1) * P, :])
                    w_sb[nm, k] = t_

            for t in range(NT):
                rows = slice(t * P, (t + 1) * P)
                xr_rows = x_pool.tile([P, L], F32, tag="xr_rows")
                xi_rows = x_pool.tile([P, L], F32, tag="xi_rows")
                nc.sync.dma_start(out=xr_rows[:], in_=xr_d[rows, :])
                nc.sync.dma_start(out=xi_rows[:], in_=xi_d[rows, :])

                # Transpose the token tile: xT chunks live at
                # xT[:, k*P:(k+1)*P] = x_rows[:, k*P:(k+1)*P].T
                xrT = xt_pool.tile([P, L], F32R, tag="xrT")
                xiT = xt_pool.tile([P, L], F32R, tag="xiT")
                for src, dst in ((xr_rows, xrT), (xi_rows, xiT)):
                    for g in range(2):
                        tp = ps_pool.tile([P, 4 * P], F32, tag="ps")
                        for j in range(4):
                            k = g * 4 + j
                            nc.tensor.transpose(
                                tp[:, j * P : (j + 1) * P],
                                src[:, k * P : (k + 1) * P],
                                ident[:],
                            )
                        nc.scalar.copy(dst[:, g * 4 * P : (g + 1) * 4 * P], tp[:])

                # Accumulate the four real matmul outputs.
                #   re_n = sum_k xrT_k @ wr_k[n] + xiT_k @ nwi_k[n]
                #   im_n = sum_k xrT_k @ wi_k[n] + xiT_k @ wr_k[n]
                out_sb = o_pool.tile([P, L, 2], F32, tag="out_sb")
                for n in range(2):
                    ncol = slice(n * 512, (n + 1) * 512)
                    acc_re = ps_pool.tile([P, 512], F32, tag="ps")
                    acc_im = ps_pool.tile([P, 512], F32, tag="ps")
                    for k in range(KC):
                        xrT_k = xrT[:, k * P : (k + 1) * P]
                        xiT_k = xiT[:, k * P : (k + 1) * P]
                        first = k == 0
                        last = k == KC - 1
                        nc.tensor.matmul(
                            acc_re[:], xrT_k, w_sb["wr", k][:, ncol],
                            start=first, stop=False,
                        )
                        nc.tensor.matmul(
                            acc_re[:], xiT_k, w_sb["nwi", k][:, ncol],
                            start=False, stop=last,
                        )
                        nc.tensor.matmul(
                            acc_im[:], xrT_k, w_sb["wi", k][:, ncol],
                            start=first, stop=False,
                        )
                        nc.tensor.matmul(
                            acc_im[:], xiT_k, w_sb["wr", k][:, ncol],
                            start=False, stop=last,
                        )
                    # Interleave re/im into complex64 memory order.
                    nc.vector.tensor_copy(out_sb[:, n * 512 : (n + 1) * 512, 0], acc_re[:])
                    nc.vector.tensor_copy(out_sb[:, n * 512 : (n + 1) * 512, 1], acc_im[:])

                nc.sync.dma_start(out=out_d[rows, :], in_=out_sb[:])

    nc.compile()
    return nc


def kernel(x_re: np.ndarray, x_im: np.ndarray, phases: np.ndarray) -> np.ndarray:
    global _CACHED_NC, LAST_RESULTS

    x_re = np.ascontiguousarray(x_re, dtype=np.float32)
    x_im = np.ascontiguousarray(x_im, dtype=np.float32)
    phases = np.ascontiguousarray(phases, dtype=np.float32)

    W = _build_W(phases)                      # (B, L, L) complex64
    Wr = np.ascontiguousarray(W.real, dtype=np.float32)
    Wi = np.ascontiguousarray(W.imag, dtype=np.float32)
    nWi = np.ascontiguousarray(-Wi)

    if _CACHED_NC is None:
        _CACHED_NC = _build_program()
    nc = _CACHED_NC

    half = N_TOKENS // 2
    in_maps = []
    for c in range(N_CORES):
        b, h = c // 2, c % 2
        in_maps.append(
            {
                "xr": x_re[h * half : (h + 1) * half],
                "xi": x_im[h * half : (h + 1) * half],
                "wr": Wr[b],
                "wi": Wi[b],
                "nwi": nWi[b],
            }
        )

    res = run_bass_kernel_spmd(nc, in_maps, list(range(N_CORES)), trace=TRACE)
    LAST_RESULTS = res

    out = np.empty((MESH_BATCH, N_TOKENS, L), dtype=np.complex64)
    for c in range(N_CORES):
        b, h = c // 2, c % 2
        out[b, h * half : (h + 1) * half] = (
            res.results[c]["out"].view(np.complex64).reshape(half, L)
        )
    return out


# revision 17
# speedup vs baseline: 1.0932x; 1.0399x over previous
"""Trainium2 Bass kernel for nn_BatchTrainableButterfly.

The reference applies, per mesh-batch b, a trainable butterfly network
(10 levels of phase shifters + 2x2 directional couplers with butterfly
permutations, plus a final phase layer and bit-reversals) to every token
row x[n, :].  For fixed phases the whole network is a linear map on
C^1024, so out[b] = x @ W_b with W_b = network_b(I_1024) — a 1024x1024
complex64 matrix that is cheap to build on host (O(L^2 log L) total).

Device work per core (8 cores = 4 mesh-batches x 2 token halves):
  out_half[b] = x_half @ W_b as real fp32r matmuls on TensorE:
    re = xr@Wr + xi@(-Wi),  im = xr@Wi + xi@Wr
x arrives token-major, so each 128-token tile is transposed on the PE
(L on partitions) to serve as the matmul stationary operand; results
accumulate in PSUM, are interleaved re/im into SBUF and DMA'd out as
complex64-compatible rows.
"""

import math

import numpy as np

import concourse.tile as tile
from concourse import bacc, bass, mybir
from concourse.bass_utils import run_bass_kernel_spmd
from concourse.masks import make_identity

P = 128          # partitions
L = 1024         # butterfly length
N_TOKENS = 4096
MESH_BATCH = 4
N_CORES = 8
T = (N_TOKENS * MESH_BATCH) // N_CORES  # 2048 token-rows per core
NT = T // P      # 16 token tiles per core
KC = L // P      # 8 contraction chunks
NLEV = int(math.log2(L))  # 10

F32 = mybir.dt.float32
F32R = mybir.dt.float32r

TRACE = False
LAST_RESULTS = None

# ----------------------------------------------------------------------
# Host side: build the per-batch transfer matrices from the phases.
# ----------------------------------------------------------------------


def _bitrev(n):
    m = int(math.log2(n))
    perm = np.arange(n).reshape(n, 1)
    for _ in range(m):
        n1 = perm.shape[0] // 2
        perm = np.hstack((perm[:n1], perm[n1:]))
    return perm.squeeze(0)


def _forward_indices(length):
    idx = []
    ar = np.arange(length)
    for level in range(int(math.log2(length)) - 1):
        bs = 2 ** (level + 2)
        ind = ar.reshape(-1, length // bs, 2, bs // 2).transpose(0, 1, 3, 2)
        idx.append(ind.reshape(-1))
    return idx


def _build_W(phases):
    """phases (B, NLEV+1, L//2, 2) -> W (B, L, L) complex64 with out = x @ W."""
    B = phases.shape[0]
    br = _bitrev(L)
    fidx = _forward_indices(L)
    dc = np.array([[1.0, 1.0j], [1.0j, 1.0]], dtype=np.complex64)

    x = np.broadcast_to(np.eye(L, dtype=np.complex64), (B, L, L)).copy()
    x = x[..., br]
    for level in range(NLEV):
        x = x.reshape(B, L, L // 2, 2)
        ph = phases[:, level : level + 1, :, :]            # (B, 1, L//2, 2)
        x = x * np.exp(1j * ph.astype(np.complex64))
        x = x @ dc
        x = x.reshape(B, L, L)
        if level < NLEV - 1:
            x = x[..., fidx[level]]
    ph = phases[:, NLEV - 1 : NLEV, :, :].reshape(B, 1, L)
    x = x * np.exp(1j * ph.astype(np.complex64))
    x = x[..., br]
    return (x / np.float32(np.sqrt(L))).astype(np.complex64)


# ----------------------------------------------------------------------
# Device side: complex matmul kernel (SPMD, one (batch, half) per core).
# ----------------------------------------------------------------------

_CACHED_NC = None


def _build_program():
    nc = bacc.Bacc(
        "TRN2", target_bir_lowering=False, debug=False, num_devices=N_CORES
    )

    xr_d = nc.declare_dram_parameter("xr", [T, L], F32, isOutput=False)
    xi_d = nc.declare_dram_parameter("xi", [T, L], F32, isOutput=False)
    wr_d = nc.declare_dram_parameter("wr", [L, L], F32R, isOutput=False)
    wi_d = nc.declare_dram_parameter("wi", [L, L], F32R, isOutput=False)
    out_d = nc.declare_dram_parameter("out", [T, 2 * L], F32, isOutput=True)

    with tile.TileContext(nc) as tc:
        with (
            tc.tile_pool(name="const", bufs=1) as const_pool,
            tc.tile_pool(name="w", bufs=1) as w_pool,
            tc.tile_pool(name="x", bufs=3) as x_pool,
            tc.tile_pool(name="xt", bufs=2) as xt_pool,
            tc.tile_pool(name="osb", bufs=3) as o_pool,
            tc.tile_pool(name="ps", bufs=8, space=bass.MemorySpace.PSUM) as ps_pool,
        ):
            ident = const_pool.tile([P, P], F32)
            make_identity(nc, ident[:])

            # Warm the PE HAM while W streams in: dummy transposes keep the
            # tensor engine busy >3.4us so it reaches full clock before the
            # real matmuls start.
            warm = ps_pool.tile([P, 4 * P], F32, tag="ps")
            for _ in range(12):
                for j in range(4):
                    nc.tensor.transpose(
                        warm[:, j * P : (j + 1) * P], ident[:], ident[:]
                    )

            # Stream W into SBUF once: per k-chunk tiles (P x L), natural layout
            # (partition = contraction row within chunk, free = output column).
            # k-major order so the first token tile's accumulation can start
            # after only a few chunks have landed.
            w_sb = {}
            for k in range(KC):
                for nm, dram in (("wr", wr_d), ("wi", wi_d)):
                    t_ = w_pool.tile([P, L], F32R, tag=f"{nm}{k}")
                    nc.sync.dma_start(out=t_[:], in_=dram[k * P : (k + 1) * P, :])
                    w_sb[nm, k] = t_
                # -Wi derived on device: saves a third of the W stream, which
                # gates the kernel head while PE waits on weights.
                nwi = w_pool.tile([P, L], F32R, tag=f"nwi{k}")
                nc.vector.tensor_scalar_mul(nwi[:], w_sb["wi", k][:], -1.0)
                w_sb["nwi", k] = nwi

            for t in range(NT):
                rows = slice(t * P, (t + 1) * P)
                xr_rows = x_pool.tile([P, L], F32, tag="xr_rows")
                xi_rows = x_pool.tile([P, L], F32, tag="xi_rows")
                nc.sync.dma_start(out=xr_rows[:], in_=xr_d[rows, :])
                nc.sync.dma_start(out=xi_rows[:], in_=xi_d[rows, :])

                # Transpose the token tile: xT chunks live at
                # xT[:, k*P:(k+1)*P] = x_rows[:, k*P:(k+1)*P].T
                xrT = xt_pool.tile([P, L], F32R, tag="xrT")
                xiT = xt_pool.tile([P, L], F32R, tag="xiT")
                for src, dst in ((xr_rows, xrT), (xi_rows, xiT)):
                    for g in range(2):
                        tp = ps_pool.tile([P, 4 * P], F32, tag="ps")
                        for j in range(4):
                            k = g * 4 + j
                            nc.tensor.transpose(
                                tp[:, j * P : (j + 1) * P],
                                src[:, k * P : (k + 1) * P],
                                ident[:],
                            )
                        nc.scalar.copy(dst[:, g * 4 * P : (g + 1) * 4 * P], tp[:])

                # Accumulate the four real matmul outputs.
                #   re_n = sum_k xrT_k @ wr_k[n] + xiT_k @ nwi_k[n]
                #   im_n = sum_k xrT_k @ wi_k[n] + xiT_k @ wr_k[n]
                out_sb = o_pool.tile([P, L, 2], F32, tag="out_sb")
                for n in range(2):
                    ncol = slice(n * 512, (n + 1) * 512)
                    acc_re = ps_pool.tile([P, 512], F32, tag="ps")
                    acc_im = ps_pool.tile([P, 512], F32, tag="ps")
                    for k in range(KC):
                        xrT_k = xrT[:, k * P : (k + 1) * P]
                        xiT_k = xiT[:, k * P : (k + 1) * P]
                        first = k == 0
                        last = k == KC - 1
                        nc.tensor.matmul(
                            acc_re[:], xrT_k, w_sb["wr", k][:, ncol],
                            start=first, stop=False,
                        )
                        nc.tensor.matmul(
                            acc_re[:], xiT_k, w_sb["nwi", k][:, ncol],
                            start=False, stop=last,
                        )
                        nc.tensor.matmul(
                            acc_im[:], xrT_k, w_sb["wi", k][:, ncol],
                            start=first, stop=False,
                        )
                        nc.tensor.matmul(
                            acc_im[:], xiT_k, w_sb["wr", k][:, ncol],
                            start=False, stop=last,
                        )
                    # Interleave re/im into complex64 memory order.
                    nc.vector.tensor_copy(out_sb[:, n * 512 : (n + 1) * 512, 0], acc_re[:])
                    nc.vector.tensor_copy(out_sb[:, n * 512 : (n + 1) * 512, 1], acc_im[:])

                nc.sync.dma_start(out=out_d[rows, :], in_=out_sb[:])

    nc.compile()
    return nc


def kernel(x_re: np.ndarray, x_im: np.ndarray, phases: np.ndarray) -> np.ndarray:
    global _CACHED_NC, LAST_RESULTS

    x_re = np.ascontiguousarray(x_re, dtype=np.float32)
    x_im = np.ascontiguousarray(x_im, dtype=np.float32)
    phases = np.ascontiguousarray(phases, dtype=np.float32)

    W = _build_W(phases)                      # (B, L, L) complex64
    Wr = np.ascontiguousarray(W.real, dtype=np.float32)
    Wi = np.ascontiguousarray(W.imag, dtype=np.float32)

    if _CACHED_NC is None:
        _CACHED_NC = _build_program()
    nc = _CACHED_NC

    half = N_TOKENS // 2
    in_maps = []
    for c in range(N_CORES):
        b, h = c // 2, c % 2
        in_maps.append(
            {
                "xr": x_re[h * half : (h + 1) * half],
                "xi": x_im[h * half : (h + 1) * half],
                "wr": Wr[b],
                "wi": Wi[b],
            }
        )

    res = run_bass_kernel_spmd(nc, in_maps, list(range(N_CORES)), trace=TRACE)
    LAST_RESULTS = res

    out = np.empty((MESH_BATCH, N_TOKENS, L), dtype=np.complex64)
    for c in range(N_CORES):
        b, h = c // 2, c % 2
        out[b, h * half : (h + 1) * half] = (
            res.results[c]["out"].view(np.complex64).reshape(half, L)
        )
    return out
